# revision 30
# baseline (speedup 1.0000x reference)
"""Trainium2 Bass kernel for BasicAttentionModule (pooled attention + residual).

Computation (per sample): 8x8 avg-pool -> 1x1-conv q/k/v over 1024 tokens ->
softmax attention -> nearest 8x upsample -> residual add.

Sharding (v4, DMA-saturating pipeline): 4 cores per sample quad; core i
handles quarter r=i%4 (64 rows) of samples 2q and 2q+1 (q=i//4).  DMA is
the roofline (67MB f32 in+out per core at 360GB/s ~ 187us); the schedule
keeps the SWDGE ring FIFO [loads A][loads B][stores A][stores B] saturated:

  loads          : 16 slab cast-DMAs (f32->bf16) into a persistent SBUF
                   cache, issued up front.
  pool (DVE)     : all-bf16 pairwise tree + reduce (2x 16-bit DVE mode),
                   ~2.4us/slab so tokens trail the DMA by <1 slab.
  gather A       : fp8 4-way AllGather right after slice-A pooling (~58us),
                   hidden under slice-B loads.
  attention      : bf16 on the PE, tokens-on-partitions, E=exp(energy^T),
                   denominator via ones matmul; PE kept at high pstate by
                   dependency-spread warm-up matmuls.
  residual       : ACT upsamples os along w (v-bias folded in via the
                   activation bias); DVE adds one [c,2,8,W] stride-0-row
                   broadcast per slab (innermost packed -> 2x mode), then
                   SWDGE cast-store bf16->f32.  Slice-B pooling and
                   slice-A residual are hand-interleaved on the DVE so the
                   slice-B gather fires by ~110us and slice-B stores are
                   ready before the ring drains slice-A stores.
"""

import ml_dtypes
import numpy as np

import concourse.bass as bass
import concourse.mybir as mybir
import concourse.tile as tile
from concourse.bass_utils import run_bass_kernel_spmd

F32 = mybir.dt.float32
BF16 = mybir.dt.bfloat16
AF = mybir.ActivationFunctionType

B, C, H, W = 4, 256, 256, 256
S = 8                      # pool stride
KCH = 32                   # key channels
N_CORES = 8
NS = 2                     # time slices (samples per core)
HQ = H // 4                # 64 rows per core per slice
HP, WP = HQ // S, W // S   # 8 x 32 pooled grid per core slice
NTQ = HP * WP              # 256 tokens owned per core per slice
NTOK = 4 * NTQ             # 1024 tokens per sample
CCH = C // 128             # 2 channel chunks
NJ = NTOK // 128           # 8 token chunks (m on partitions)
TS = HP // 2               # 4 slabs (of 16 rows) per channel chunk per slice

_CACHE: dict = {}


def _split_multi_waits(nc):
    """walrus in this container accepts at most ONE sync-wait per
    instruction; hoist extra waits onto inserted NoOps (same engine,
    right before the instruction -> identical semantics)."""
    import json

    d = json.loads(mybir.module_to_json_string(nc.m))
    n = 0
    for fn in d["functions"]:
        for bb in fn["blocks"]:
            out = []
            for inst in bb.get("instructions", []):
                si = inst.get("sync_info")
                waits = (si or {}).get("on_wait") or []
                if len(waits) > 1:
                    for w in waits[:-1]:
                        n += 1
                        out.append({
                            "debug": inst.get("debug", 0),
                            "engine": inst["engine"],
                            "ins": [], "outs": [],
                            "name": f"I-wsplit-{n}",
                            "opcode": "NoOp",
                            "sync_info": {"on_update": [], "on_wait": [w]},
                        })
                    si["on_wait"] = [waits[-1]]
                out.append(inst)
            bb["instructions"] = out
    nc.m = mybir.module_from_json_string(json.dumps(d))
    return n


def _build(split_waits=True):
    nc = bass.Bass(num_devices=N_CORES)

    xh = nc.declare_dram_parameter("xh", [NS, C, HQ, W], BF16, isOutput=False)
    qw = nc.declare_dram_parameter("qw", [CCH, 128, KCH], BF16, isOutput=False)
    kw = nc.declare_dram_parameter("kw", [CCH, 128, KCH], BF16, isOutput=False)
    vw = nc.declare_dram_parameter("vw", [CCH, 128, C], BF16, isOutput=False)
    qb = nc.declare_dram_parameter("qb", [KCH], F32, isOutput=False)
    kb = nc.declare_dram_parameter("kb", [KCH], F32, isOutput=False)
    vb = nc.declare_dram_parameter("vb", [CCH, 128], F32, isOutput=False)
    out = nc.declare_dram_parameter("out", [NS, C, HQ, W], BF16, isOutput=True)

    with tile.TileContext(nc) as tc:
        with (
            tc.tile_pool(name="const", bufs=1) as constp,
            tc.tile_pool(name="wtree", bufs=2) as wtp,
            tc.tile_pool(name="attn", bufs=1) as attnp,
            tc.tile_pool(name="cache", bufs=1) as cachep,
            tc.tile_pool(name="pqk", bufs=1, space="PSUM") as pqk,
            tc.tile_pool(name="pe", bufs=1, space="PSUM") as pep,
            tc.tile_pool(name="pacc", bufs=1, space="PSUM") as pacc,
            tc.tile_pool(name="dram", bufs=1, space="DRAM") as dram,
        ):
            # ---- constants / weights ----
            qw_sb = [constp.tile([128, KCH], BF16, name=f"qw{k}") for k in range(CCH)]
            kw_sb = [constp.tile([128, KCH], BF16, name=f"kw{k}") for k in range(CCH)]
            vw_sb = [constp.tile([128, C], BF16, name=f"vw{k}") for k in range(CCH)]
            qb_sb = constp.tile([KCH, 1], F32, name="qb")
            kb_sb = constp.tile([KCH, 1], F32, name="kb")
            vb_sb = [constp.tile([128, 1], F32, name=f"vb{k}") for k in range(CCH)]
            for k in range(CCH):
                nc.scalar.dma_start(out=qw_sb[k][:], in_=qw[k])
                nc.scalar.dma_start(out=kw_sb[k][:], in_=kw[k])
                nc.scalar.dma_start(out=vw_sb[k][:], in_=vw[k])
                nc.scalar.dma_start(out=vb_sb[k][:], in_=vb[k])
            nc.scalar.dma_start(out=qb_sb[:], in_=qb[:])
            nc.scalar.dma_start(out=kb_sb[:], in_=kb[:])
            ones128 = constp.tile([128, 128], BF16, name="ones128")
            nc.vector.memset(ones128[:], 1.0)
            # pre-warm the ACT function table off the critical path (the
            # first activation otherwise pays a 1.3us ACT_TABLE_LOAD mid-
            # attention chain)
            actwarm = constp.tile([128, 1], F32, name="actwarm")
            nc.scalar.activation(actwarm[:], ones128[:, :1], AF.Exp)

            xf_own = [[constp.tile([128, 2 * NTQ], BF16, name=f"xfo{s}_{k}")
                       for k in range(CCH)] for s in range(NS)]
            for s in range(NS):
                for k in range(CCH):
                    nc.vector.memset(xf_own[s][k][:, NTQ:], 0.0)
            xf_full = [[constp.tile([128, NTOK], BF16, name=f"xff{s}_{k}")
                        for k in range(CCH)] for s in range(NS)]
            xf8_own = [[constp.tile([128, NTQ], mybir.dt.float8e4,
                                    name=f"x8o{s}_{k}")
                        for k in range(CCH)] for s in range(NS)]
            xf8_full = [[constp.tile([128, NTOK], mybir.dt.float8e4,
                                     name=f"x8f{s}_{k}")
                        for k in range(CCH)] for s in range(NS)]
            # w-upsampled attention rows (v-bias folded in), tag-shared
            # across slices (slice-B write waits slice-A residual reads)
            osw_sb = [attnp.tile([128, HP, W], BF16, tag=f"osw{k}",
                                 name=f"osw{k}")
                      for k in range(CCH)]

            cc_in = [dram.tile([C, NTQ], mybir.dt.float8e4, name=f"cc_in{s}")
                     for s in range(NS)]
            cc_out = [dram.tile([4, C, NTQ], mybir.dt.float8e4,
                                name=f"cc_out{s}")
                      for s in range(NS)]
            groups = [[0, 1, 2, 3], [4, 5, 6, 7]]

            # ---- slab load helper: SWDGE ring order is
            # [loads A][loads B][stores A][stores B]; collective triggers
            # are interleaved between load-gen groups on the gpsimd
            # sequencer (desc-gen is ring-space-gated, so a trigger emitted
            # after all 16 gens would stall until ~66us) ----
            cached_tiles = {}

            def load_slab(s, k, t):
                cs = slice(k * 128, (k + 1) * 128)
                rs = slice(t * 16, (t + 1) * 16)
                feat = cachep.tile([128, 16, W], BF16,
                                   tag=f"cc{s}_{k}_{t}", name="cfeat")
                cached_tiles[(s, k, t)] = feat
                nc.gpsimd.dma_start(out=feat[:], in_=xh[s, cs, rs, :])

            # ---- PE warm-ups: spread across the load phase via data deps
            # on arriving slabs (keeps the PE pstate up without ever
            # blocking the attention chain) ----
            psum_warm = pqk.tile([KCH, NTOK], F32, tag="qk", name="psum_warm")

            def warm_on(view, n):
                for _ in range(n):
                    nc.tensor.matmul(psum_warm[:, :512], qw_sb[0][:], view,
                                     start=True, stop=True)

            # ---- pooling: all-bf16 pairwise tree (DVE 2x 16-bit mode) ----
            def pool_slab(s, k, t):
                feat = cached_tiles[(s, k, t)]
                with nc.allow_low_precision(reason="bf16 pool tree"):
                    l1 = wtp.tile([128, 8, W], BF16, tag="l1", name="l1")
                    nc.vector.tensor_add(l1[:], feat[:, 0::2, :],
                                         feat[:, 1::2, :])
                    l2 = wtp.tile([128, 4, W], BF16, tag="l2", bufs=1,
                                  name="l2")
                    nc.vector.tensor_add(l2[:], l1[:, 0::2, :], l1[:, 1::2, :])
                    l3 = wtp.tile([128, 2, W], BF16, tag="l3", bufs=1,
                                  name="l3")
                    nc.vector.tensor_add(l3[:], l2[:, 0::2, :], l2[:, 1::2, :])
                    dst = xf_own[s][k][:, t * 2 * WP : (t + 1) * 2 * WP]
                    dst = dst.rearrange("c (i wp) -> c i wp", i=2)
                    nc.vector.reduce_sum(
                        dst, l3.rearrange("c i (wp r) -> c i wp r", r=S),
                        axis=mybir.AxisListType.X)

            def cast_chunk(s, k):
                with nc.allow_low_precision(reason="fp8 exchange"):
                    nc.vector.tensor_copy(xf8_own[s][k][:],
                                          xf_own[s][k][:, :NTQ])

            # ---- per-slice gather.  Each chunk's cc_in send is issued on
            # the SP engine the moment its cast lands (the transfer crawls
            # ~12us behind the load traffic, so an early start is the only
            # lever); trigger on gpsimd; receive half on the ACT engine ----
            def gather_send_chunk(s, k):
                nc.sync.dma_start(out=cc_in[s][k * 128:(k + 1) * 128, :],
                                  in_=xf8_own[s][k][:])

            def gather_coll(s):
                nc.gpsimd.collective_compute(
                    "AllGather", mybir.AluOpType.bypass,
                    replica_groups=groups,
                    ins=[cc_in[s].opt()], outs=[cc_out[s].opt()],
                )

            def gather_recv(s):
                for k in range(CCH):
                    src = cc_out[s][:, k * 128:(k + 1) * 128, :].rearrange(
                        "g c t -> c g t")
                    dstf = xf8_full[s][k].rearrange("c (g t) -> c g t", g=4)
                    nc.scalar.dma_start(out=dstf, in_=src)
                for k in range(CCH):
                    with nc.allow_low_precision(reason="fp8 exchange"):
                        nc.scalar.activation(xf_full[s][k][:],
                                             xf8_full[s][k][:], AF.Copy)

            # ---- per-slice attention (front half: everything up to the
            # normalizer; os is produced per channel chunk so each chunk's
            # upsample/residual/store can flow without the other) ----
            def attention_front(s):
                # q projection (own tokens only), bias on the ACT evac
                q_sb = attnp.tile([KCH, NTQ], BF16, name=f"q{s}")
                psum_q = pqk.tile([KCH, NTOK], F32, tag="qk", name="psum_q")
                for k in range(CCH):
                    nc.tensor.matmul(psum_q[:, :2 * NTQ], qw_sb[k][:],
                                     xf_own[s][k][:],
                                     start=(k == 0), stop=(k == CCH - 1))
                with nc.allow_low_precision(reason="bf16 attention"):
                    nc.scalar.activation(q_sb[:], psum_q[:, :NTQ], AF.Identity,
                                         bias=qb_sb[:])

                # k projection over the gathered tokens (chunk-outer so the
                # k0 half runs as soon as its gather lands)
                k_sb = attnp.tile([KCH, NTOK], BF16, name=f"k{s}")
                psum_k = pqk.tile([KCH, NTOK], F32, tag="qk", name="psum_k")
                for k in range(CCH):
                    for mh in range(2):
                        ms = slice(mh * 512, (mh + 1) * 512)
                        nc.tensor.matmul(psum_k[:, ms], kw_sb[k][:],
                                         xf_full[s][k][:, ms],
                                         start=(k == 0), stop=(k == CCH - 1))
                with nc.allow_low_precision(reason="bf16 attention"):
                    nc.scalar.activation(k_sb[:], psum_k[:], AF.Identity,
                                         bias=kb_sb[:])

                # energies E = exp(energy^T), denominator interleaved
                e_sb = [attnp.tile([128, NTQ], BF16, name=f"e{s}_{j}")
                        for j in range(NJ)]
                psum_den = pacc.tile([128, NTQ], F32, tag="den", name="psum_den",
                                     padded_shape=[128, 512])
                pending_den = []
                for j in range(NJ):
                    js = slice(j * 128, (j + 1) * 128)
                    psum_e = pep.tile([128, NTQ], F32, tag="pe", bufs=2,
                                      name="psum_e", padded_shape=[128, 512])
                    nc.tensor.matmul(psum_e[:], k_sb[:, js], q_sb[:],
                                     start=True, stop=True)
                    nc.scalar.activation(e_sb[j][:], psum_e[:], AF.Exp)
                    pending_den.append(j)
                    if j >= 1:
                        jd = pending_den.pop(0)
                        nc.tensor.matmul(psum_den[:], ones128[:], e_sb[jd][:],
                                         start=(jd == 0), stop=False)
                for jd in pending_den:
                    nc.tensor.matmul(psum_den[:], ones128[:], e_sb[jd][:],
                                     start=False, stop=(jd == NJ - 1))

                # v^T (tokens on partitions)
                vt_sb = [attnp.tile([128, C], BF16, name=f"vt{s}_{j}")
                         for j in range(NJ)]
                for j in range(NJ):
                    js = slice(j * 128, (j + 1) * 128)
                    psum_vt = pep.tile([128, C], F32, tag="pvt", bufs=2,
                                       name="psum_vt", padded_shape=[128, 512])
                    for k in range(CCH):
                        nc.tensor.matmul(psum_vt[:], xf_full[s][k][:, js],
                                         vw_sb[k][:],
                                         start=(k == 0), stop=(k == CCH - 1))
                    with nc.allow_low_precision(reason="bf16 attention"):
                        nc.scalar.activation(vt_sb[j][:], psum_vt[:], AF.Copy)

                recip = attnp.tile([128, NTQ], F32, name=f"recip{s}")
                nc.vector.reciprocal(recip[:], psum_den[:])
                return e_sb, vt_sb, recip

            def os_chunk(s, k, front):
                # one channel chunk of the attention output: PE os psums,
                # DVE normalize, ACT w-upsample + v-bias (softmax rows sum
                # to 1, so bias-after-normalize is exact)
                e_sb, vt_sb, recip = front
                os = attnp.tile([128, NTQ], F32, name=f"os{s}_{k}")
                psum_os = pacc.tile([128, NTQ], F32, tag="pos", bufs=1,
                                    name="psum_os", padded_shape=[128, 512])
                for j in range(NJ):
                    nc.tensor.matmul(psum_os[:],
                                     vt_sb[j][:, k * 128:(k + 1) * 128],
                                     e_sb[j][:],
                                     start=(j == 0), stop=(j == NJ - 1))
                nc.vector.tensor_mul(os[:], psum_os[:], recip[:])
                src = bass.AP(os.tensor, os.offset,
                              [list(os.ap[0]), [WP, HP], [1, WP], [0, S]])
                dst = osw_sb[k][:].rearrange("c hp (wp wr) -> c hp wp wr",
                                             wr=S)
                with nc.allow_low_precision(reason="bf16 residual"):
                    nc.scalar.activation(dst, src, AF.Identity,
                                         bias=vb_sb[k][:])

            def resid_slab(s, k, t):
                # one DVE add per slab: cache[c,2,8,W] += osw rows (stride-0
                # row broadcast; innermost packed bf16 -> 2x mode), then
                # SWDGE cast-store
                feat = cached_tiles[(s, k, t)]
                f4 = feat[:].rearrange("c (g r) w -> c g r w", g=2)
                osw = osw_sb[k]
                src = bass.AP(osw.tensor, osw.offset + t * 2 * W,
                              [list(osw.ap[0]), [W, 2], [0, 8], [1, W]])
                with nc.allow_low_precision(reason="bf16 residual"):
                    nc.vector.tensor_add(f4, f4, src)
                cs = slice(k * 128, (k + 1) * 128)
                rs = slice(t * 16, (t + 1) * 16)
                nc.gpsimd.dma_start(out=out[s, cs, rs, :], in_=feat[:])

            # ---- emission schedule.  bf16 I/O makes the DMA cheap (47us
            # of loads+stores); the critical path is now the gather ->
            # attention -> residual chain, so: issue all loads, pool both
            # slices as slabs arrive, one AllGather per slice (pipelined on
            # the CC core), and keep the PE warm via dependency-spread
            # matmuls so attention runs at speed when its tokens land ----
            # gpsimd ring order: [A slabs x8][B-k0 slabs x4][cc_in A x2]
            # collA [B-k1 slabs x4][cc_in B x2] collB [store gens].  The
            # cc_in descriptors land just behind the B-k0 slabs and execute
            # the moment the engines reach them (~47us) -> collective A
            # triggers ~49us instead of ~58.
            for k in range(CCH):
                for t in range(TS):
                    load_slab(0, k, t)
            for t in range(TS):
                load_slab(1, 0, t)

            # PE warm-ups: spread across slice-A slab arrivals
            for k in range(CCH):
                for t in range(TS):
                    v = cached_tiles[(0, k, t)].rearrange("c h w -> c (h w)")
                    warm_on(v[:, :512], 8)

            # slice A pool; each chunk's send is issued as soon as that
            # chunk's tokens are cast
            for t in range(TS):
                load_slab(1, 1, t)
            for k in range(CCH):
                for t in range(TS):
                    pool_slab(0, k, t)
                cast_chunk(0, k)
                gather_send_chunk(0, k)
            gather_coll(0)

            for k in range(CCH):
                for t in range(TS):
                    v = cached_tiles[(1, k, t)].rearrange("c h w -> c (h w)")
                    warm_on(v[:, :512], 6)

            # slice B pool + gather (mesh queues behind slice A's on CC)
            for k in range(CCH):
                for t in range(TS):
                    pool_slab(1, k, t)
                cast_chunk(1, k)
                gather_send_chunk(1, k)
            gather_coll(1)

            gather_recv(0)

            # keep the PE clock up until the gathered tokens land; the last
            # group reads the freshly-decoded tokens so the ramp reaches
            # right into the attention matmuls
            warm_on(xf_own[1][0][:, :512], 8)
            warm_on(xf_own[1][1][:, :512], 8)
            warm_on(xf_full[0][0][:, :512], 10)

            # DVE order mirrors the store ring order [A-k0][A-k1][B-k0]
            # [B-k1] so every store slab is residual-complete just ahead of
            # its ring slot; recv(1) sits between the A chunks on the ACT
            # so slice-B's decode is done before its attention needs it
            front_a = attention_front(0)
            os_chunk(0, 0, front_a)
            for t in range(TS):
                resid_slab(0, 0, t)
            os_chunk(0, 1, front_a)
            gather_recv(1)
            for t in range(TS):
                resid_slab(0, 1, t)

            front_b = attention_front(1)
            os_chunk(1, 0, front_b)
            for t in range(TS):
                resid_slab(1, 0, t)
            os_chunk(1, 1, front_b)
            for t in range(TS):
                resid_slab(1, 1, t)

    if split_waits:
        _split_multi_waits(nc)
    return nc


def _get_nc():
    if "nc" not in _CACHE:
        _CACHE["nc"] = _build()
    return _CACHE["nc"]


def kernel(features, q_w, q_b, k_w, k_b, v_w, v_b):
    nc = _get_nc()
    inv = 1.0 / (S * S)
    scale = float(KCH) ** -0.5
    qw_eff = np.ascontiguousarray(
        (q_w.T * (scale * inv)).astype(ml_dtypes.bfloat16).reshape(CCH, 128, KCH))
    qb_eff = np.ascontiguousarray((q_b * scale).astype(np.float32))
    kw_eff = np.ascontiguousarray(
        (k_w.T * inv).astype(ml_dtypes.bfloat16).reshape(CCH, 128, KCH))
    kb_eff = np.ascontiguousarray(k_b.astype(np.float32))
    vw_eff = np.ascontiguousarray(
        (v_w.T * inv).astype(ml_dtypes.bfloat16).reshape(CCH, 128, C))
    vb_eff = np.ascontiguousarray(v_b.astype(np.float32).reshape(CCH, 128))

    features = np.asarray(features, dtype=np.float32).astype(ml_dtypes.bfloat16)
    in_maps = []
    for i in range(N_CORES):
        q, r = i // 4, i % 4
        rs = slice(r * HQ, (r + 1) * HQ)
        in_maps.append({
            "xh": np.ascontiguousarray(
                np.stack([features[2 * q, :, rs, :],
                          features[2 * q + 1, :, rs, :]])),
            "qw": qw_eff, "kw": kw_eff, "vw": vw_eff,
            "qb": qb_eff, "kb": kb_eff, "vb": vb_eff,
        })

    res = run_bass_kernel_spmd(nc, in_maps, list(range(N_CORES)))
    out = np.empty((B, C, H, W), dtype=np.float32)
    for i in range(N_CORES):
        q, r = i // 4, i % 4
        rs = slice(r * HQ, (r + 1) * HQ)
        out[2 * q, :, rs, :] = res.results[i]["out"][0].astype(np.float32)
        out[2 * q + 1, :, rs, :] = res.results[i]["out"][1].astype(np.float32)
    return out


# revision 32
# speedup vs baseline: 2.2692x; 2.2692x over previous
"""Trainium2 Bass kernel for BasicAttentionModule (pooled attention + residual).

Computation (per sample): 8x8 avg-pool -> 1x1-conv q/k/v over 1024 tokens ->
softmax attention -> nearest 8x upsample -> residual add.

Sharding (v4, DMA-saturating pipeline): 4 cores per sample quad; core i
handles quarter r=i%4 (64 rows) of samples 2q and 2q+1 (q=i//4).  DMA is
the roofline (67MB f32 in+out per core at 360GB/s ~ 187us); the schedule
keeps the SWDGE ring FIFO [loads A][loads B][stores A][stores B] saturated:

  loads          : 16 slab cast-DMAs (f32->bf16) into a persistent SBUF
                   cache, issued up front.
  pool (DVE)     : all-bf16 pairwise tree + reduce (2x 16-bit DVE mode),
                   ~2.4us/slab so tokens trail the DMA by <1 slab.
  gather A       : fp8 4-way AllGather right after slice-A pooling (~58us),
                   hidden under slice-B loads.
  attention      : bf16 on the PE, tokens-on-partitions, E=exp(energy^T),
                   denominator via ones matmul; PE kept at high pstate by
                   dependency-spread warm-up matmuls.
  residual       : ACT upsamples os along w (v-bias folded in via the
                   activation bias); DVE adds one [c,2,8,W] stride-0-row
                   broadcast per slab (innermost packed -> 2x mode), then
                   SWDGE cast-store bf16->f32.  Slice-B pooling and
                   slice-A residual are hand-interleaved on the DVE so the
                   slice-B gather fires by ~110us and slice-B stores are
                   ready before the ring drains slice-A stores.
"""

import ml_dtypes
import numpy as np

import concourse.bass as bass
import concourse.mybir as mybir
import concourse.tile as tile
from concourse.bass_utils import run_bass_kernel_spmd

F32 = mybir.dt.float32
BF16 = mybir.dt.bfloat16
AF = mybir.ActivationFunctionType

B, C, H, W = 4, 256, 256, 256
S = 8                      # pool stride
KCH = 32                   # key channels
N_CORES = 8
NS = 2                     # time slices (samples per core)
HQ = H // 4                # 64 rows per core per slice
HP, WP = HQ // S, W // S   # 8 x 32 pooled grid per core slice
NTQ = HP * WP              # 256 tokens owned per core per slice
NTOK = 4 * NTQ             # 1024 tokens per sample
CCH = C // 128             # 2 channel chunks
NJ = NTOK // 128           # 8 token chunks (m on partitions)
TS = HP // 2               # 4 slabs (of 16 rows) per channel chunk per slice

_CACHE: dict = {}


def _split_multi_waits(nc):
    """walrus in this container accepts at most ONE sync-wait per
    instruction; hoist extra waits onto inserted NoOps (same engine,
    right before the instruction -> identical semantics)."""
    import json

    d = json.loads(mybir.module_to_json_string(nc.m))
    n = 0
    for fn in d["functions"]:
        for bb in fn["blocks"]:
            out = []
            for inst in bb.get("instructions", []):
                si = inst.get("sync_info")
                waits = (si or {}).get("on_wait") or []
                if len(waits) > 1:
                    for w in waits[:-1]:
                        n += 1
                        out.append({
                            "debug": inst.get("debug", 0),
                            "engine": inst["engine"],
                            "ins": [], "outs": [],
                            "name": f"I-wsplit-{n}",
                            "opcode": "NoOp",
                            "sync_info": {"on_update": [], "on_wait": [w]},
                        })
                    si["on_wait"] = [waits[-1]]
                out.append(inst)
            bb["instructions"] = out
    nc.m = mybir.module_from_json_string(json.dumps(d))
    return n


def _build(split_waits=True):
    nc = bass.Bass(num_devices=N_CORES)

    xh = nc.declare_dram_parameter("xh", [NS, C, HQ, W], BF16, isOutput=False)
    qw = nc.declare_dram_parameter("qw", [CCH, 128, KCH], BF16, isOutput=False)
    kw = nc.declare_dram_parameter("kw", [CCH, 128, KCH], BF16, isOutput=False)
    vw = nc.declare_dram_parameter("vw", [CCH, 128, C], BF16, isOutput=False)
    qb = nc.declare_dram_parameter("qb", [KCH], F32, isOutput=False)
    kb = nc.declare_dram_parameter("kb", [KCH], F32, isOutput=False)
    vb = nc.declare_dram_parameter("vb", [CCH, 128], F32, isOutput=False)
    out = nc.declare_dram_parameter("out", [NS, C, HQ, W], BF16, isOutput=True)

    with tile.TileContext(nc) as tc:
        with (
            tc.tile_pool(name="const", bufs=1) as constp,
            tc.tile_pool(name="wtree", bufs=2) as wtp,
            tc.tile_pool(name="attn", bufs=1) as attnp,
            tc.tile_pool(name="cache", bufs=1) as cachep,
            tc.tile_pool(name="pqk", bufs=1, space="PSUM") as pqk,
            tc.tile_pool(name="pe", bufs=1, space="PSUM") as pep,
            tc.tile_pool(name="pacc", bufs=1, space="PSUM") as pacc,
            tc.tile_pool(name="dram", bufs=1, space="DRAM") as dram,
        ):
            # ---- constants / weights ----
            qw_sb = [constp.tile([128, KCH], BF16, name=f"qw{k}") for k in range(CCH)]
            kw_sb = [constp.tile([128, KCH], BF16, name=f"kw{k}") for k in range(CCH)]
            vw_sb = [constp.tile([128, C], BF16, name=f"vw{k}") for k in range(CCH)]
            qb_sb = constp.tile([KCH, 1], F32, name="qb")
            kb_sb = constp.tile([KCH, 1], F32, name="kb")
            vb_sb = [constp.tile([128, 1], F32, name=f"vb{k}") for k in range(CCH)]
            for k in range(CCH):
                nc.scalar.dma_start(out=qw_sb[k][:], in_=qw[k])
                nc.scalar.dma_start(out=kw_sb[k][:], in_=kw[k])
                nc.scalar.dma_start(out=vw_sb[k][:], in_=vw[k])
                nc.scalar.dma_start(out=vb_sb[k][:], in_=vb[k])
            nc.scalar.dma_start(out=qb_sb[:], in_=qb[:])
            nc.scalar.dma_start(out=kb_sb[:], in_=kb[:])
            ones128 = constp.tile([128, 128], BF16, name="ones128")
            nc.vector.memset(ones128[:], 1.0)
            # pre-warm the ACT function table off the critical path (the
            # first activation otherwise pays a 1.3us ACT_TABLE_LOAD mid-
            # attention chain)
            actwarm = constp.tile([128, 1], F32, name="actwarm")
            nc.scalar.activation(actwarm[:], ones128[:, :1], AF.Exp)

            xf_own = [[constp.tile([128, 2 * NTQ], BF16, name=f"xfo{s}_{k}")
                       for k in range(CCH)] for s in range(NS)]
            for s in range(NS):
                for k in range(CCH):
                    nc.vector.memset(xf_own[s][k][:, NTQ:], 0.0)
            xf_full = [[constp.tile([128, NTOK], BF16, name=f"xff{s}_{k}")
                        for k in range(CCH)] for s in range(NS)]
            xf8_own = [[constp.tile([128, NTQ], mybir.dt.float8e4,
                                    name=f"x8o{s}_{k}")
                        for k in range(CCH)] for s in range(NS)]
            xf8_full = [[constp.tile([128, NTOK], mybir.dt.float8e4,
                                     name=f"x8f{s}_{k}")
                        for k in range(CCH)] for s in range(NS)]
            # w-upsampled attention rows (v-bias folded in), tag-shared
            # across slices (slice-B write waits slice-A residual reads)
            osw_sb = [attnp.tile([128, HP, W], BF16, tag=f"osw{k}",
                                 name=f"osw{k}")
                      for k in range(CCH)]

            cc_in = [dram.tile([C, NTQ], mybir.dt.float8e4, name=f"cc_in{s}")
                     for s in range(NS)]
            cc_out = [dram.tile([4, C, NTQ], mybir.dt.float8e4,
                                name=f"cc_out{s}")
                      for s in range(NS)]
            groups = [[0, 1, 2, 3], [4, 5, 6, 7]]

            # ---- slab load helper: SWDGE ring order is
            # [loads A][loads B][stores A][stores B]; collective triggers
            # are interleaved between load-gen groups on the gpsimd
            # sequencer (desc-gen is ring-space-gated, so a trigger emitted
            # after all 16 gens would stall until ~66us) ----
            cached_tiles = {}

            def load_slab(s, k, t):
                cs = slice(k * 128, (k + 1) * 128)
                rs = slice(t * 16, (t + 1) * 16)
                feat = cachep.tile([128, 16, W], BF16,
                                   tag=f"cc{s}_{k}_{t}", name="cfeat")
                cached_tiles[(s, k, t)] = feat
                nc.gpsimd.dma_start(out=feat[:], in_=xh[s, cs, rs, :])

            # ---- PE warm-ups: spread across the load phase via data deps
            # on arriving slabs (keeps the PE pstate up without ever
            # blocking the attention chain) ----
            psum_warm = pqk.tile([KCH, NTOK], F32, tag="qk", name="psum_warm")

            def warm_on(view, n):
                for _ in range(n):
                    nc.tensor.matmul(psum_warm[:, :512], qw_sb[0][:], view,
                                     start=True, stop=True)

            # ---- pooling: all-bf16 pairwise tree (DVE 2x 16-bit mode) ----
            def pool_slab(s, k, t):
                feat = cached_tiles[(s, k, t)]
                with nc.allow_low_precision(reason="bf16 pool tree"):
                    l1 = wtp.tile([128, 8, W], BF16, tag="l1", name="l1")
                    nc.vector.tensor_add(l1[:], feat[:, 0::2, :],
                                         feat[:, 1::2, :])
                    l2 = wtp.tile([128, 4, W], BF16, tag="l2", bufs=1,
                                  name="l2")
                    nc.vector.tensor_add(l2[:], l1[:, 0::2, :], l1[:, 1::2, :])
                    l3 = wtp.tile([128, 2, W], BF16, tag="l3", bufs=1,
                                  name="l3")
                    nc.vector.tensor_add(l3[:], l2[:, 0::2, :], l2[:, 1::2, :])
                    dst = xf_own[s][k][:, t * 2 * WP : (t + 1) * 2 * WP]
                    dst = dst.rearrange("c (i wp) -> c i wp", i=2)
                    nc.vector.reduce_sum(
                        dst, l3.rearrange("c i (wp r) -> c i wp r", r=S),
                        axis=mybir.AxisListType.X)

            def cast_chunk(s, k):
                with nc.allow_low_precision(reason="fp8 exchange"):
                    nc.vector.tensor_copy(xf8_own[s][k][:],
                                          xf_own[s][k][:, :NTQ])

            # ---- per-slice gather.  Send half on the SP engine, collective
            # trigger on gpsimd, receive half on the ACT engine -- three
            # engines so a recv wait can never block the next send ----
            def gather_send(s):
                for k in range(CCH):
                    nc.sync.dma_start(out=cc_in[s][k * 128:(k + 1) * 128, :],
                                      in_=xf8_own[s][k][:])

            def gather_coll(s):
                nc.gpsimd.collective_compute(
                    "AllGather", mybir.AluOpType.bypass,
                    replica_groups=groups,
                    ins=[cc_in[s].opt()], outs=[cc_out[s].opt()],
                )

            def gather_recv(s):
                for k in range(CCH):
                    src = cc_out[s][:, k * 128:(k + 1) * 128, :].rearrange(
                        "g c t -> c g t")
                    dstf = xf8_full[s][k].rearrange("c (g t) -> c g t", g=4)
                    nc.scalar.dma_start(out=dstf, in_=src)
                for k in range(CCH):
                    with nc.allow_low_precision(reason="fp8 exchange"):
                        nc.scalar.activation(xf_full[s][k][:],
                                             xf8_full[s][k][:], AF.Copy)

            # ---- per-slice attention (front half: everything up to the
            # normalizer; os is produced per channel chunk so each chunk's
            # upsample/residual/store can flow without the other) ----
            def attention_front(s):
                # q projection (own tokens only), bias on the ACT evac
                q_sb = attnp.tile([KCH, NTQ], BF16, name=f"q{s}")
                psum_q = pqk.tile([KCH, NTOK], F32, tag="qk", name="psum_q")
                for k in range(CCH):
                    nc.tensor.matmul(psum_q[:, :2 * NTQ], qw_sb[k][:],
                                     xf_own[s][k][:],
                                     start=(k == 0), stop=(k == CCH - 1))
                with nc.allow_low_precision(reason="bf16 attention"):
                    nc.scalar.activation(q_sb[:], psum_q[:, :NTQ], AF.Identity,
                                         bias=qb_sb[:])

                # k projection over the gathered tokens (chunk-outer so the
                # k0 half runs as soon as its gather lands)
                k_sb = attnp.tile([KCH, NTOK], BF16, name=f"k{s}")
                psum_k = pqk.tile([KCH, NTOK], F32, tag="qk", name="psum_k")
                for k in range(CCH):
                    for mh in range(2):
                        ms = slice(mh * 512, (mh + 1) * 512)
                        nc.tensor.matmul(psum_k[:, ms], kw_sb[k][:],
                                         xf_full[s][k][:, ms],
                                         start=(k == 0), stop=(k == CCH - 1))
                with nc.allow_low_precision(reason="bf16 attention"):
                    nc.scalar.activation(k_sb[:], psum_k[:], AF.Identity,
                                         bias=kb_sb[:])

                # energies E = exp(energy^T), denominator interleaved
                e_sb = [attnp.tile([128, NTQ], BF16, name=f"e{s}_{j}")
                        for j in range(NJ)]
                psum_den = pacc.tile([128, NTQ], F32, tag="den", name="psum_den",
                                     padded_shape=[128, 512])
                pending_den = []
                for j in range(NJ):
                    js = slice(j * 128, (j + 1) * 128)
                    psum_e = pep.tile([128, NTQ], F32, tag="pe", bufs=2,
                                      name="psum_e", padded_shape=[128, 512])
                    nc.tensor.matmul(psum_e[:], k_sb[:, js], q_sb[:],
                                     start=True, stop=True)
                    nc.scalar.activation(e_sb[j][:], psum_e[:], AF.Exp)
                    pending_den.append(j)
                    if j >= 1:
                        jd = pending_den.pop(0)
                        nc.tensor.matmul(psum_den[:], ones128[:], e_sb[jd][:],
                                         start=(jd == 0), stop=False)
                for jd in pending_den:
                    nc.tensor.matmul(psum_den[:], ones128[:], e_sb[jd][:],
                                     start=False, stop=(jd == NJ - 1))

                # v^T (tokens on partitions)
                vt_sb = [attnp.tile([128, C], BF16, name=f"vt{s}_{j}")
                         for j in range(NJ)]
                for j in range(NJ):
                    js = slice(j * 128, (j + 1) * 128)
                    psum_vt = pep.tile([128, C], F32, tag="pvt", bufs=2,
                                       name="psum_vt", padded_shape=[128, 512])
                    for k in range(CCH):
                        nc.tensor.matmul(psum_vt[:], xf_full[s][k][:, js],
                                         vw_sb[k][:],
                                         start=(k == 0), stop=(k == CCH - 1))
                    with nc.allow_low_precision(reason="bf16 attention"):
                        nc.scalar.activation(vt_sb[j][:], psum_vt[:], AF.Copy)

                recip = attnp.tile([128, NTQ], F32, name=f"recip{s}")
                nc.vector.reciprocal(recip[:], psum_den[:])
                return e_sb, vt_sb, recip

            def os_chunk(s, k, front):
                # one channel chunk of the attention output: PE os psums,
                # DVE normalize, ACT w-upsample + v-bias (softmax rows sum
                # to 1, so bias-after-normalize is exact)
                e_sb, vt_sb, recip = front
                os = attnp.tile([128, NTQ], F32, name=f"os{s}_{k}")
                psum_os = pacc.tile([128, NTQ], F32, tag="pos", bufs=1,
                                    name="psum_os", padded_shape=[128, 512])
                for j in range(NJ):
                    nc.tensor.matmul(psum_os[:],
                                     vt_sb[j][:, k * 128:(k + 1) * 128],
                                     e_sb[j][:],
                                     start=(j == 0), stop=(j == NJ - 1))
                nc.vector.tensor_mul(os[:], psum_os[:], recip[:])
                src = bass.AP(os.tensor, os.offset,
                              [list(os.ap[0]), [WP, HP], [1, WP], [0, S]])
                dst = osw_sb[k][:].rearrange("c hp (wp wr) -> c hp wp wr",
                                             wr=S)
                with nc.allow_low_precision(reason="bf16 residual"):
                    nc.scalar.activation(dst, src, AF.Identity,
                                         bias=vb_sb[k][:])

            def resid_slab(s, k, t):
                # one DVE add per slab: cache[c,2,8,W] += osw rows (stride-0
                # row broadcast; innermost packed bf16 -> 2x mode), then
                # SWDGE cast-store
                feat = cached_tiles[(s, k, t)]
                f4 = feat[:].rearrange("c (g r) w -> c g r w", g=2)
                osw = osw_sb[k]
                src = bass.AP(osw.tensor, osw.offset + t * 2 * W,
                              [list(osw.ap[0]), [W, 2], [0, 8], [1, W]])
                with nc.allow_low_precision(reason="bf16 residual"):
                    nc.vector.tensor_add(f4, f4, src)
                cs = slice(k * 128, (k + 1) * 128)
                rs = slice(t * 16, (t + 1) * 16)
                nc.gpsimd.dma_start(out=out[s, cs, rs, :], in_=feat[:])

            # ---- emission schedule.  bf16 I/O makes the DMA cheap (47us
            # of loads+stores); the critical path is now the gather ->
            # attention -> residual chain, so: issue all loads, pool both
            # slices as slabs arrive, one AllGather per slice (pipelined on
            # the CC core), and keep the PE warm via dependency-spread
            # matmuls so attention runs at speed when its tokens land ----
            # gpsimd ring order: [A slabs x8][B-k0 slabs x4][cc_in A x2]
            # collA [B-k1 slabs x4][cc_in B x2] collB [store gens].  The
            # cc_in descriptors land just behind the B-k0 slabs and execute
            # the moment the engines reach them (~47us) -> collective A
            # triggers ~49us instead of ~58.
            for k in range(CCH):
                for t in range(TS):
                    load_slab(0, k, t)
            for t in range(TS):
                load_slab(1, 0, t)

            # PE warm-ups: spread across slice-A slab arrivals
            for k in range(CCH):
                for t in range(TS):
                    v = cached_tiles[(0, k, t)].rearrange("c h w -> c (h w)")
                    warm_on(v[:, :512], 8)

            # remaining loads + warm-ups, then slice A pool + gather
            for t in range(TS):
                load_slab(1, 1, t)
            for k in range(CCH):
                for t in range(TS):
                    v = cached_tiles[(1, k, t)].rearrange("c h w -> c (h w)")
                    warm_on(v[:, :512], 6)

            for k in range(CCH):
                for t in range(TS):
                    pool_slab(0, k, t)
                cast_chunk(0, k)
            gather_send(0)
            gather_coll(0)

            # slice B pool + gather (mesh queues behind slice A's on CC)
            for k in range(CCH):
                for t in range(TS):
                    pool_slab(1, k, t)
                cast_chunk(1, k)
            gather_send(1)
            gather_coll(1)

            gather_recv(0)

            # keep the PE clock up until the gathered tokens land; the last
            # group reads the freshly-decoded tokens so the ramp reaches
            # right into the attention matmuls
            warm_on(xf_own[1][0][:, :512], 8)
            warm_on(xf_own[1][1][:, :512], 8)
            warm_on(xf_full[0][0][:, :512], 10)

            # DVE order mirrors the store ring order [A-k0][A-k1][B-k0]
            # [B-k1] so every store slab is residual-complete just ahead of
            # its ring slot; recv(1) sits between the A chunks on the ACT
            # so slice-B's decode is done before its attention needs it
            front_a = attention_front(0)
            os_chunk(0, 0, front_a)
            for t in range(TS):
                resid_slab(0, 0, t)
            os_chunk(0, 1, front_a)
            gather_recv(1)
            for t in range(TS):
                resid_slab(0, 1, t)

            front_b = attention_front(1)
            os_chunk(1, 0, front_b)
            for t in range(TS):
                resid_slab(1, 0, t)
            os_chunk(1, 1, front_b)
            for t in range(TS):
                resid_slab(1, 1, t)

    if split_waits:
        _split_multi_waits(nc)
    return nc


def _get_nc():
    if "nc" not in _CACHE:
        _CACHE["nc"] = _build()
    return _CACHE["nc"]


def kernel(features, q_w, q_b, k_w, k_b, v_w, v_b):
    nc = _get_nc()
    inv = 1.0 / (S * S)
    scale = float(KCH) ** -0.5
    qw_eff = np.ascontiguousarray(
        (q_w.T * (scale * inv)).astype(ml_dtypes.bfloat16).reshape(CCH, 128, KCH))
    qb_eff = np.ascontiguousarray((q_b * scale).astype(np.float32))
    kw_eff = np.ascontiguousarray(
        (k_w.T * inv).astype(ml_dtypes.bfloat16).reshape(CCH, 128, KCH))
    kb_eff = np.ascontiguousarray(k_b.astype(np.float32))
    vw_eff = np.ascontiguousarray(
        (v_w.T * inv).astype(ml_dtypes.bfloat16).reshape(CCH, 128, C))
    vb_eff = np.ascontiguousarray(v_b.astype(np.float32).reshape(CCH, 128))

    features = np.asarray(features, dtype=np.float32).astype(ml_dtypes.bfloat16)
    in_maps = []
    for i in range(N_CORES):
        q, r = i // 4, i % 4
        rs = slice(r * HQ, (r + 1) * HQ)
        in_maps.append({
            "xh": np.ascontiguousarray(
                np.stack([features[2 * q, :, rs, :],
                          features[2 * q + 1, :, rs, :]])),
            "qw": qw_eff, "kw": kw_eff, "vw": vw_eff,
            "qb": qb_eff, "kb": kb_eff, "vb": vb_eff,
        })

    res = run_bass_kernel_spmd(nc, in_maps, list(range(N_CORES)))
    out = np.empty((B, C, H, W), dtype=np.float32)
    for i in range(N_CORES):
        q, r = i // 4, i % 4
        rs = slice(r * HQ, (r + 1) * HQ)
        out[2 * q, :, rs, :] = res.results[i]["out"][0].astype(np.float32)
        out[2 * q + 1, :, rs, :] = res.results[i]["out"][1].astype(np.float32)
    return out
